# revision 1
# baseline (speedup 1.0000x reference)
"""Low-rank multi-head attention Bass kernel for Trainium2 (8 NeuronCores).

Sharding: (batch, query-block) data parallel. 8 cores = 2 batches x 4 query
blocks. Each core receives the full sequence of its batch, np.roll'ed so its
own query block sits at rows 0:SQ. It computes k1/v1 only for its own
query slice and AllGathers the slices within each 4-core batch group, then
runs attention + output projection for its SQ queries.

Math (per core, per head h):
  q1T = Wq1 @ xq.T            [R, SQ]
  k1T = Wk1 @ xb.T            [R, S]   (k1aug row 32 = ones)
  v1T = Wv1 @ xb.T            [R, S]
  wm_aug = [Wm[h]; b2[h]]     qh rows 0:32 = Wm q1T + b2 x ones (so the k-side
                              bias sum_t k1T[t,j] b2[t] emerges from the k1T
                              contraction), row 32 = q1.b1 + b3
  scoresT[j,i] = k1aug[:,j].T @ qh[:,i]   (K=33; all four bias terms inside)
  attnT = exp(0.125*scoresT)              (no max-subtraction; scores O(1);
                                           no ACT bias -> exp batched per pair)
  aT_aug[65, SQ] = [Vh_h | ones].T @ attnT   (row 64 = softmax denominator)
  wvT[h*64+d, i] = aT[d,i] * (1/denominator[i])
  outT = Wo2T_aug.T @ [o1T; ones], with bv/bo folded into the aug row.

All matmuls run as float32r (1 col/cycle on the PE vs 4 for strict fp32).
"""

import sys

sys.path.insert(0, "/opt/trn_rl_repo")

from contextlib import ExitStack

import numpy as np

import concourse.bass as bass
import concourse.tile as tile
from concourse import bacc
from concourse import mybir
from concourse.masks import make_identity

F32 = mybir.dt.float32
AF = mybir.ActivationFunctionType

H, D, R, N = 20, 64, 32, 1280
NCORES = 8
QP = 4  # query blocks per batch
SCALE = float(D) ** -0.5  # 0.125


def _chunks(total, size):
    out = []
    s = 0
    while s < total:
        out.append((s, min(size, total - s)))
        s += size
    return out


def build_nc(S, SQ, phase=4):
    nc = bacc.Bacc("TRN2", target_bir_lowering=False, debug=False, num_devices=NCORES)

    xb = nc.dram_tensor("xb", [S, N], F32, kind="ExternalInput")
    Wq1 = nc.dram_tensor("Wq1", [R, N], F32, kind="ExternalInput")
    Wq2 = nc.dram_tensor("Wq2", [N, R], F32, kind="ExternalInput")
    bq = nc.dram_tensor("bq", [N], F32, kind="ExternalInput")
    Wk1 = nc.dram_tensor("Wk1", [R, N], F32, kind="ExternalInput")
    Wk2 = nc.dram_tensor("Wk2", [N, R], F32, kind="ExternalInput")
    bk = nc.dram_tensor("bk", [N], F32, kind="ExternalInput")
    Wv1 = nc.dram_tensor("Wv1", [R, N], F32, kind="ExternalInput")
    Wv2 = nc.dram_tensor("Wv2", [N, R], F32, kind="ExternalInput")
    bv = nc.dram_tensor("bv", [N], F32, kind="ExternalInput")
    Wo1 = nc.dram_tensor("Wo1", [R, N], F32, kind="ExternalInput")
    Wo2 = nc.dram_tensor("Wo2", [N, R], F32, kind="ExternalInput")
    bo = nc.dram_tensor("bo", [N], F32, kind="ExternalInput")
    out = nc.dram_tensor("out", [SQ, N], F32, kind="ExternalOutput")

    SCH = _chunks(S, 128)  # sequence chunks (j)
    JSUB = _chunks(S, 512)  # projection free-dim chunks
    OSUB = _chunks(N, 512)  # out-proj free-dim chunks
    ICH = _chunks(SQ, 128)  # output row chunks
    NJ = len(SCH)
    SQP = SQ + (SQ % 2)  # f32r matmuls need an even moving free-dim

    F32R = mybir.dt.float32r

    def evac(dst, src):
        nc.vector.tensor_copy(dst, src)

    eev = [0]

    def evac_early(dst, src):
        # during the early phase ACT is idle; split psum evacuations
        eev[0] += 1
        if eev[0] % 2 == 0:
            nc.scalar.copy(dst, src)
        else:
            nc.vector.tensor_copy(dst, src)

    def mm(out_, lhsT, rhs, **kw):
        nc.tensor.matmul(out_, lhsT, rhs, **kw)

    def tr(out_, in_, ident_ap):
        nc.tensor.matmul(out_, in_, ident_ap, is_transpose=True)

    with tile.TileContext(nc) as tc, ExitStack() as ctx:
        wp = ctx.enter_context(tc.tile_pool(name="wp", bufs=1))
        ps_sc = ctx.enter_context(tc.tile_pool(name="ps_sc", bufs=2, space="PSUM"))
        attn_p = ctx.enter_context(tc.tile_pool(name="attn_p", bufs=6))
        small_p = ctx.enter_context(tc.tile_pool(name="small_p", bufs=4))
        outp = ctx.enter_context(tc.tile_pool(name="outp", bufs=2))
        psum_ctx = ExitStack()
        ps = psum_ctx.enter_context(tc.tile_pool(name="ps_e", bufs=4, space="PSUM"))

        # ---- persistent SBUF tensors ----
        ident = wp.tile([128, 128], F32)
        make_identity(nc, ident[:])
        ones128 = wp.tile([128, 64], F32)
        nc.gpsimd.memset(ones128[:], 1.0)
        onesS = wp.tile([1, S], F32)
        nc.gpsimd.memset(onesS[:], 1.0)
        onesP = wp.tile([128, 20], F32)
        nc.gpsimd.memset(onesP[:], 1.0)

        WqT = wp.tile([128, 320], F32R)  # Wq1.T chunk c at cols 32c:32c+32
        WkT = wp.tile([128, 320], F32R)
        WvT = wp.tile([128, 320], F32R)
        WoT = wp.tile([128, 320], F32R)
        Wq2aug = wp.tile([128, 330], F32)  # [Wq2 | bq] per 64-row head slice
        Wk2aug = wp.tile([128, 330], F32)
        bk_c = wp.tile([128, 10], F32)
        bv_c = wp.tile([128, 12], F32)
        bo_row = wp.tile([1, N], F32)
        Wv2T = wp.tile([32, N], F32R)
        Wo2Ta = wp.tile([33, N], F32R)  # rows 0:32 Wo2.T, row 32 = Wo2@Wo1@bv + bo
        q1Ta = wp.tile([33, SQP], F32R)  # rows 0:32 q1T, row 32 ones
        k1aug = wp.tile([33, S], F32R)  # rows 0:32 k1T, row 32 ones
        v1T = wp.tile([32, S], F32R)
        o1a = wp.tile([33, SQP], F32R)
        t1sb = wp.tile([32, 2], F32R)

        wvT = [wp.tile([128, SQP], F32R, name=f"wvT{c}", tag=f"wvT{c}") for c in range(10)]

        # ================= early phase (xT + projections) =================
        with tc.tile_pool(name="early", bufs=1) as ep, tc.tile_pool(
            name="xin_p", bufs=4
        ) as xin_p, tc.tile_pool(name="wload", bufs=3) as wload:
            # --- k/v weights first: the AllGather should dispatch ASAP ---
            for (w_dram, dstT) in ((Wk1, WkT), (Wv1, WvT)):
                wsb = wload.tile([32, N], F32, tag="wsb")
                nc.sync.dma_start(wsb[:], w_dram[:])
                tp = ps.tile([128, 320], F32, tag="ps")
                for c in range(10):
                    tr(
                        tp[:, 32 * c : 32 * c + 32],
                        wsb[:, 128 * c : 128 * c + 128],
                        ident[:32, :32],
                    )
                evac_early(dstT[:], tp[:])

            # --- x load + transpose into xT (feature-major, queries only;
            # k1/v1 for the rest of the sequence arrive via AllGather) ---
            QCH = _chunks(SQ, 128)
            xT = [ep.tile([128, SQP], F32R, name=f"xT{c}", tag=f"xT{c}") for c in range(10)]
            for g in range(0, len(QCH), 4):
                grp = QCH[g : g + 4]
                xins = []
                for (s0, p) in grp:
                    xin = xin_p.tile([128, N], F32, tag="xin")
                    nc.sync.dma_start(xin[:p, :], xb[s0 : s0 + p, :])
                    xins.append((xin, s0, p))
                for c in range(10):
                    wtot = sum(p for (_, _, p) in xins)
                    tp3 = ps.tile([128, 512], F32, tag="ps")
                    col = 0
                    for (xin, s0, p) in xins:
                        tr(
                            tp3[:, col : col + p],
                            xin[:p, 128 * c : 128 * c + 128],
                            ident[:p, :p],
                        )
                        col += p
                    evac_early(xT[c][:, grp[0][0] : grp[0][0] + wtot], tp3[:, :wtot])

            # local k1/v1 slices (this core's SQ keys), stacked [64, SQP]
            kv_sb = wload.tile([64, SQP], F32R, tag="kv_sb")
            for row0, wT in ((0, WkT), (32, WvT)):
                pps = ps.tile([32, SQP], F32, tag="ps")
                for c in range(10):
                    mm(
                        pps[:],
                        wT[:, 32 * c : 32 * c + 32],
                        xT[c][:],
                        start=(c == 0),
                        stop=(c == 9),
                    )
                evac_early(kv_sb[row0 : row0 + 32, :], pps[:])

            # AllGather the [64, SQ] k/v slices within each 4-core batch
            # group -> full-sequence k1T / v1T in original row order. All
            # remaining weight prep below overlaps the collective latency.
            with tc.tile_pool(name="dramp", bufs=1, space="DRAM") as dramp:
                cc_in = dramp.tile([64, SQ], F32R)
                cc_out = dramp.tile([64 * QP, SQ], F32R)
                nc.sync.dma_start(cc_in[:], kv_sb[:, 0:SQ])
                nc.gpsimd.collective_compute(
                    "AllGather",
                    mybir.AluOpType.bypass,
                    replica_groups=[
                        list(range(g * QP, (g + 1) * QP))
                        for g in range(NCORES // QP)
                    ],
                    ins=[cc_in[:].opt()],
                    outs=[cc_out[:].opt()],
                )
                nc.sync.dma_start(
                    k1aug[0:32, 0:S].rearrange("p (r s) -> p r s", r=QP),
                    cc_out[:].rearrange("(r x) s -> x r s", x=64)[0:32],
                )
                nc.sync.dma_start(
                    v1T[0:32, 0:S].rearrange("p (r s) -> p r s", r=QP),
                    cc_out[:].rearrange("(r x) s -> x r s", x=64)[32:64],
                )

            # --- remaining weights (overlap the collective) ---
            for (w_dram, dstT) in ((Wq1, WqT), (Wo1, WoT)):
                wsb = wload.tile([32, N], F32, tag="wsb")
                nc.sync.dma_start(wsb[:], w_dram[:])
                tp = ps.tile([128, 320], F32, tag="ps")
                for c in range(10):
                    tr(
                        tp[:, 32 * c : 32 * c + 32],
                        wsb[:, 128 * c : 128 * c + 128],
                        ident[:32, :32],
                    )
                evac_early(dstT[:], tp[:])

            for (w_dram, dst) in ((Wq2, Wq2aug), (Wk2, Wk2aug)):
                nc.sync.dma_start(
                    dst[:, :].rearrange("p (c r) -> p c r", r=33)[:, :, 0:32],
                    w_dram[:].rearrange("(c p) r -> p c r", p=128),
                )
            nc.sync.dma_start(
                Wq2aug[:, :].rearrange("p (c r) -> p c r", r=33)[:, :, 32:33],
                bq[:].rearrange("(c p) -> p c", p=128).unsqueeze(2),
            )
            nc.sync.dma_start(
                Wk2aug[:, :].rearrange("p (c r) -> p c r", r=33)[:, :, 32:33],
                bk[:].rearrange("(c p) -> p c", p=128).unsqueeze(2),
            )
            nc.sync.dma_start(bk_c[:], bk[:].rearrange("(c p) -> p c", p=128))
            nc.gpsimd.memset(bv_c[:], 0.0)
            nc.sync.dma_start(bv_c[:, 0:10], bv[:].rearrange("(c p) -> p c", p=128))
            nc.sync.dma_start(bo_row[:], bo[:].unsqueeze(0))

            # Wv2 / Wo2: load [128, 320] (chunk-major), PE-transpose to [32, N]
            for (w_dram, dstT) in ((Wv2, Wv2T), (Wo2, Wo2Ta)):
                wsb2 = wload.tile([128, 320], F32, tag="wsb2")
                nc.sync.dma_start(
                    wsb2[:].rearrange("p (c r) -> p c r", r=32),
                    w_dram[:].rearrange("(c p) r -> p c r", p=128),
                )
                for g0 in range(0, 10, 4):
                    gn = min(4, 10 - g0)
                    tp2 = ps.tile([32, 512], F32, tag="ps")
                    for k in range(gn):
                        c = g0 + k
                        tr(
                            tp2[:, 128 * k : 128 * k + 128],
                            wsb2[:, 32 * c : 32 * c + 32],
                            ident[:],
                        )
                    evac_early(dstT[0:32, 128 * g0 : 128 * (g0 + gn)], tp2[:, : 128 * gn])

            # --- q1 projection ---
            q1ps = ps.tile([32, SQP], F32, tag="ps")
            for c in range(10):
                mm(
                    q1ps[:],
                    WqT[:, 32 * c : 32 * c + 32],
                    xT[c][:, 0:SQP],
                    start=(c == 0),
                    stop=(c == 9),
                )
            evac_early(q1Ta[0:32, :], q1ps[:])
            nc.vector.tensor_copy(q1Ta[32:33, :], onesS[:, 0:SQP])
            nc.vector.tensor_copy(k1aug[32:33, :], onesS[:])

            # --- bo_eff into Wo2Ta row 32 ---
            bv_cr = wload.tile([128, 12], F32R, tag="bv_cr")
            nc.vector.tensor_copy(bv_cr[:], bv_c[:])
            t1ps = ps.tile([32, 2], F32, tag="ps")
            for c in range(10):
                mm(
                    t1ps[:],
                    WoT[:, 32 * c : 32 * c + 32],
                    bv_cr[:, c : c + 2],
                    start=(c == 0),
                    stop=(c == 9),
                )
            evac(t1sb[:], t1ps[:])
            for (n0, nw) in OSUB:
                beps = ps.tile([1, 512], F32, tag="ps")
                mm(beps[:, :nw], t1sb[:, 0:1], Wo2Ta[0:32, n0 : n0 + nw])
                nc.vector.tensor_add(
                    Wo2Ta[32:33, n0 : n0 + nw], beps[:, :nw], bo_row[:, n0 : n0 + nw]
                )

        # ================= Vh_aug construction =================
        if phase < 1:
            nc.sync.dma_start(out[0:128, :], q1Ta[0:33, 0:N] if False else k1aug[0:33, 0:N])
        late = ctx.enter_context(tc.tile_pool(name="late", bufs=1))
        Vh = [
            late.tile([128, H * 65], F32R, name=f"Vh{j}", tag=f"Vh{j}")
            for j in range(NJ)
        ]
        for j, (j0, p) in enumerate(SCH):
            if phase < 2:
                break
            ones_ap = Vh[j][:p, :].rearrange("p (h c) -> p h c", c=65)[:, :, 64:65]
            nc.vector.tensor_copy(ones_ap, onesP[:p, :].unsqueeze(2))
            for (n0, nw) in OSUB:
                vps = ps.tile([128, 512], F32, tag="ps")
                mm(vps[:p, :nw], v1T[:, j0 : j0 + p], Wv2T[:, n0 : n0 + nw])
                h0 = n0 // 64
                hn = nw // 64
                dst = Vh[j][:p, 65 * h0 : 65 * (h0 + hn)].rearrange(
                    "p (h c) -> p h c", c=65
                )[:, :, 0:64]
                src = vps[:p, :nw].rearrange("p (h c) -> p h c", c=64)
                evac(dst, src)

        # ---- per-head constants, hoisted out of the attention loop ----
        qh_all = []
        for h in range(H):
            hc, half = h // 2, (h % 2) * 64
            # wm_aug [33, 33]:
            #   cols 0:32: rows 0:32 = Wm[h] = Wq2h.T @ Wk2h, row 32 = b2[h]
            #   col 32   = [b1; b3]  (b1 = Wq2h.T bk_h, b3 = bq.bk)
            # One f32r matmul against q1Ta then yields all 33 qh rows at
            # partition 0 (f32r matmuls reject partition-offset outputs).
            wmps = ps.tile([33, 33], F32, tag="ps")
            mm(
                wmps[0:32, 0:32],
                Wq2aug[half : half + 64, 33 * hc : 33 * hc + 32],
                Wk2aug[half : half + 64, 33 * hc : 33 * hc + 32],
            )
            mm(
                wmps[32:33, 0:32],
                Wq2aug[half : half + 64, 33 * hc + 32 : 33 * hc + 33],
                Wk2aug[half : half + 64, 33 * hc : 33 * hc + 32],
                skip_group_check=True,
            )
            mm(
                wmps[0:33, 32:33],
                Wq2aug[half : half + 64, 33 * hc : 33 * hc + 33],
                bk_c[half : half + 64, hc : hc + 1],
                skip_group_check=True,
            )
            wm = small_p.tile([33, 33], F32R, tag="wm")
            evac(wm[:], wmps[:])

            # qh: rows 0:32 = Wm q1T + b2 x ones, row 32 = q1.b1 + b3
            qhps = ps.tile([33, SQP], F32, tag="ps")
            mm(qhps[:], wm[:], q1Ta[:])
            qh = wp.tile([33, SQP], F32R, name=f"qh{h}", tag=f"qh{h}")
            evac(qh[:], qhps[:])
            qh_all.append(qh)

        # ---- switch PSUM pools: early pool out, 4 accumulator banks in ----
        psum_ctx.close()
        psum_ctx = ExitStack()
        ps_acc = psum_ctx.enter_context(
            tc.tile_pool(name="ps_acc", bufs=1, space="PSUM")
        )

        # ================= attention core (head pairs) =================
        for hp in range(H // 2 if phase >= 3 else 0):
            heads = (2 * hp, 2 * hp + 1)
            accs = [
                ps_acc.tile([65, SQP], F32, tag="acc", name=f"acc{h}", bufs=3)
                for h in heads
            ]

            # software-pipelined: scores(j+1) issues on the PE before
            # attnV(j), so the PE never idles waiting on exp(j)
            scs = {}

            def do_scores(j):
                j0, p = SCH[j]
                sc = ps_sc.tile([128, 1024], F32, tag="sc", name=f"sc{j}")
                mm(sc[:p, 0:SQP], k1aug[:, j0 : j0 + p], qh_all[heads[0]][:])
                mm(sc[:p, 512 : 512 + SQP], k1aug[:, j0 : j0 + p], qh_all[heads[1]][:])
                scs[j] = sc

            do_scores(0)
            for j, (j0, p) in enumerate(SCH):
                sc = scs.pop(j)
                at2 = attn_p.tile([128, 2 * SQP], F32R, tag="at")
                nc.scalar.activation(
                    at2[:p, :].rearrange("p (b c) -> p b c", c=SQP),
                    sc[:p, :].rearrange("p (b c) -> p b c", c=512)[:, :, 0:SQP],
                    AF.Exp,
                    scale=SCALE,
                )
                if j + 1 < NJ:
                    do_scores(j + 1)
                for idx, h in enumerate(heads):
                    mm(
                        accs[idx][:],
                        Vh[j][:p, 65 * h : 65 * h + 65],
                        at2[:p, idx * SQP : (idx + 1) * SQP],
                        start=(j == 0),
                        stop=(j == NJ - 1),
                    )

            # normalize: wvT rows [64h:64h+64] = acc[0:64] * (1/acc[64])
            for idx, h in enumerate(heads):
                acc = accs[idx]
                half = (h % 2) * 64
                rrs = small_p.tile([65, SQP], F32, tag="rrs")
                nc.vector.reciprocal(rrs[64:65, :], acc[64:65, :])
                bc = ps_acc.tile([64, SQP], F32, tag="bc", bufs=1)
                mm(bc[:], ones128[64:65, :], rrs[64:65, :])
                bc_sb = small_p.tile([64, SQP], F32, tag="bc_sb")
                nc.vector.tensor_copy(bc_sb[:], bc[:])
                nc.vector.tensor_mul(
                    wvT[h // 2][half : half + 64, :], bc_sb[:], acc[0:64, :]
                )

        # ================= output projection =================
        psum_ctx.close()
        psum_ctx = ExitStack()
        ps = psum_ctx.enter_context(tc.tile_pool(name="ps_f", bufs=3, space="PSUM"))
        if phase < 4:
            psum_ctx.close()
            nc.compile()
            return nc
        o1ps = ps.tile([32, SQP], F32, tag="ps")
        for c in range(10):
            mm(
                o1ps[:],
                WoT[:, 32 * c : 32 * c + 32],
                wvT[c][:],
                start=(c == 0),
                stop=(c == 9),
            )
        evac_early(o1a[0:32, :], o1ps[:])
        nc.vector.tensor_copy(o1a[32:33, :], onesS[:, 0:SQP])

        for (i0, iw) in ICH:
            osb = outp.tile([128, N], F32, tag="osb")
            for (n0, nw) in OSUB:
                fps = ps.tile([128, 512], F32, tag="ps")
                mm(fps[:iw, :nw], o1a[:, i0 : i0 + iw], Wo2Ta[:, n0 : n0 + nw])
                evac_early(osb[:iw, n0 : n0 + nw], fps[:iw, :nw])
            nc.sync.dma_start(out[i0 : i0 + iw, :], osb[:iw, :])
        psum_ctx.close()

    nc.compile()
    return nc


_NC_CACHE = {}


def _get_nc(S, SQ):
    key = (S, SQ)
    if key not in _NC_CACHE:
        _NC_CACHE[key] = build_nc(S, SQ)
    return _NC_CACHE[key]


def kernel(**inputs):
    from concourse.bass_utils import run_bass_kernel_spmd

    x = np.asarray(inputs["x"], dtype=np.float32)
    B, S, n = x.shape
    assert n == N and B * QP == NCORES
    SQ = S // QP
    nc = _get_nc(S, SQ)

    wnames = [
        "Wq1", "Wq2", "bq", "Wk1", "Wk2", "bk",
        "Wv1", "Wv2", "bv", "Wo1", "Wo2", "bo",
    ]
    weights = {k: np.ascontiguousarray(np.asarray(inputs[k], dtype=np.float32)) for k in wnames}

    in_maps = []
    for core in range(NCORES):
        b, qi = divmod(core, QP)
        xbv = np.ascontiguousarray(np.roll(x[b], -SQ * qi, axis=0))
        m = {"xb": xbv}
        m.update(weights)
        in_maps.append(m)

    res = run_bass_kernel_spmd(nc, in_maps, core_ids=list(range(NCORES)))
    outs = res.results if hasattr(res, "results") else res

    out = np.zeros((B, S, N), dtype=np.float32)
    for core in range(NCORES):
        b, qi = divmod(core, QP)
        out[b, SQ * qi : SQ * (qi + 1), :] = outs[core]["out"]
    return out



# revision 19
# speedup vs baseline: 1.3017x; 1.3017x over previous
"""Low-rank multi-head attention Bass kernel for Trainium2 (8 NeuronCores).

Sharding: (batch, query-block) data parallel. 8 cores = 2 batches x 4 query
blocks. Each core receives the full (column-rolled) feature-major xT of its
batch and computes full-sequence k1/v1 locally (no collective), plus q1 for
its own 375-query block.

Host-side prep (numpy, inside kernel()):
  - xT = x[b].T rolled so the core's query block is at cols 0:375, padded
    with 36 zero columns to 1536, cast bf16.
  - All weight-derived constants: per-head Wm/b1/b2/b3 folded into wmT,
    Wv2 head layout + ones column (softmax denominator), Wo2.T with
    bo_eff = Wo2@Wo1@bv + bo row (folds both bv and bo), W*1 transposed
    into PE-stationary chunk layout.

Device math (per core, head pair hp = heads 2hp, 2hp+1):
  k1aug = [Wk1 @ xT ; ones]          [33, 1536] bf16 (zero phantom tail)
  v1Ta  = [Wv1 @ xT ; ones]          [33, 1536] bf16
  q1Ta  = [Wq1 @ xT[:, 0:376]; ones] [33, 376]  f32r
  qh_h  = wm_h.T @ q1Ta              [33, 376]  bf16 (all 4 bias terms in)
  scT   = k1aug[:, jblk].T @ qh_h    [128, 376] per j-chunk (bf16 matmul)
  at2   = exp(0.125 * scT)           fp8e4, laid out [128, 2(j), 2(h), 376]
  acc_h = sum_jj DoubleRow(Vh[jj], at2)   [65, 376] f32 (row 64 = denom)
  wvT   = acc[0:64] * (1/acc[64])    bf16 (PE broadcast of reciprocal row)
  o1a   = [Wo1 @ wvT ; ones]         [33, 376] f32r
  out   = o1a.T @ Wo2Ta              (bv/bo folded in row 32)

exp runs exclusively on ACT (the critical path ~97us); evacuations are
spread over DVE and Pool (gpsimd); attnV uses fp8 DoubleRow matmuls
(0.5 cycles/row, K=256 over two j-chunks at once).
"""

import sys

sys.path.insert(0, "/opt/trn_rl_repo")

from contextlib import ExitStack

import numpy as np

import concourse.bass as bass
import concourse.tile as tile
from concourse import bacc
from concourse import mybir

F32 = mybir.dt.float32
F32R = mybir.dt.float32r
BF16 = mybir.dt.bfloat16
FP8 = mybir.dt.float8e4
AF = mybir.ActivationFunctionType
DR = mybir.MatmulPerfMode.DoubleRow

H, D, R, N = 20, 64, 32, 1280
NCORES = 8
QP = 4  # query blocks per batch
SCALE = float(D) ** -0.5  # 0.125

# fp8 DoubleRow for the attn @ V matmuls (0.5 cycles/row, K=256)
ATTNV_DR = True


def build_nc(S):
    SQ = S // QP          # 375
    SQP = SQ + (SQ % 2)   # 376
    NJ = -(-S // 128)     # 12 j-chunks
    assert NJ % 2 == 0
    NJJ = NJ // 2
    SP = NJ * 128         # 1536 padded sequence
    NPAIR = H // 2

    nc = bacc.Bacc("TRN2", target_bir_lowering=False, debug=False, num_devices=NCORES)

    xT_d = nc.dram_tensor("xT", [N, SP], BF16, kind="ExternalInput")
    Wcat_d = nc.dram_tensor("Wcat", [128, 4 * 10 * 32], BF16, kind="ExternalInput")
    Wv2Ta_d = nc.dram_tensor("Wv2Ta", [33, H * 66], BF16, kind="ExternalInput")
    cbf_d = nc.dram_tensor("cbf", [1, SP], BF16, kind="ExternalInput")
    cf32_d = nc.dram_tensor("cf32", [1, SQP], F32R, kind="ExternalInput")
    wmT_d = nc.dram_tensor("wmT", [33, H * 33], F32R, kind="ExternalInput")
    Wo2Ta_d = nc.dram_tensor("Wo2Ta", [33, N], F32R, kind="ExternalInput")
    out_d = nc.dram_tensor("out", [SQ, N], F32, kind="ExternalOutput")

    AT2DT = FP8 if ATTNV_DR else BF16

    def mm(out_, lhsT, rhs, **kw):
        nc.tensor.matmul(out_, lhsT, rhs, **kw)

    ev = [0]

    def evac(dst, src, act_ok=False):
        # PSUM evacuations: DVE, alternating with ACT while ACT is idle
        # (GPSIMD cannot access PSUM on TRN2)
        ev[0] += 1
        if act_ok and ev[0] % 2 == 0:
            nc.scalar.copy(dst, src)
        else:
            nc.vector.tensor_copy(dst, src)

    with tile.TileContext(nc) as tc, ExitStack() as ctx:
        wp = ctx.enter_context(tc.tile_pool(name="wp", bufs=1))
        small_p = ctx.enter_context(tc.tile_pool(name="small_p", bufs=4))
        at2_p = ctx.enter_context(tc.tile_pool(name="at2_p", bufs=3))
        outp = ctx.enter_context(tc.tile_pool(name="outp", bufs=2))
        psum_ctx = ExitStack()
        psE = psum_ctx.enter_context(tc.tile_pool(name="psE", bufs=1, space="PSUM"))

        # ---- persistent SBUF tensors ----
        xTs = [wp.tile([128, SP], BF16, name=f"xT{c}", tag=f"xT{c}") for c in range(10)]
        Wc = wp.tile([128, 4 * 10 * 32], BF16)
        Wcv = Wc[:].rearrange("p (w c r) -> p w c r", w=4, c=10)
        Wv2Ta = wp.tile([33, H * 66], BF16)
        wmT = wp.tile([33, H * 33], F32R)
        Wo2Ta = wp.tile([33, N], F32R)
        k1aug = wp.tile([33, SP], BF16)
        v1Ta = wp.tile([33, SP], BF16)
        q1Ta = wp.tile([33, SQP], F32R)
        qhS = [wp.tile([33, 2 * SQP], BF16, name=f"qh{p}", tag=f"qh{p}") for p in range(NPAIR)]
        VHW = ((H * 66 + 15) // 16) * 16  # half-stride must be 16-aligned for dual-fp8 LW
        Vh = [wp.tile([128, 2 * VHW], FP8 if ATTNV_DR else BF16,
                      name=f"Vh{jj}", tag=f"Vh{jj}")
              for jj in range(NJJ if ATTNV_DR else NJ)]
        wvT = [wp.tile([128, SQP], BF16, name=f"wvT{c}", tag=f"wvT{c}") for c in range(10)]
        o1a = wp.tile([33, SQP], F32R)

        # ---- DMA in ----
        nc.sync.dma_start(Wc[:], Wcat_d[:])
        for c in range(10):
            nc.sync.dma_start(xTs[c][:], xT_d[128 * c : 128 * c + 128, :])
        nc.sync.dma_start(Wv2Ta[:], Wv2Ta_d[:])
        nc.sync.dma_start(wmT[:], wmT_d[:])
        nc.sync.dma_start(Wo2Ta[:], Wo2Ta_d[:])

        # ---- ones rows (host-staged constants) ----
        nc.sync.dma_start(k1aug[32:33, :], cbf_d[:])
        nc.sync.dma_start(v1Ta[32:33, :], cbf_d[:])
        nc.sync.dma_start(q1Ta[32:33, :], cf32_d[:])
        nc.sync.dma_start(o1a[32:33, :], cf32_d[:])

        # ================= projections =================
        # k1 full sequence: 3 sub-chunks of 512 cols, accumulated over 10
        # feature chunks; evacuate (cast bf16) as soon as each completes.
        SUBS = [(0, 512), (512, 512), (1024, 512)]

        def proj_full(pool, widx, dstT, act_ok=False):
            tiles = [pool.tile([32, 512], F32, tag=f"pj{s}", name=f"pj{widx}{s}")
                     for s in range(3)]
            for c in range(10):
                for s, (s0, sw) in enumerate(SUBS):
                    mm(tiles[s][:], Wcv[:, widx, c, :], xTs[c][:, s0 : s0 + sw],
                       start=(c == 0), stop=(c == 9))
            for s, (s0, sw) in enumerate(SUBS):
                evac(dstT[0:32, s0 : s0 + sw], tiles[s][:], act_ok=act_ok)

        proj_full(psE, 0, k1aug, act_ok=True)

        # q1 (own query block only)
        q1ps = psE.tile([32, SQP], F32, tag="q1")
        for c in range(10):
            mm(q1ps[:], Wcv[:, 2, c, :], xTs[c][:, 0:SQP], start=(c == 0), stop=(c == 9))
        nc.vector.tensor_copy(q1Ta[0:32, :], q1ps[:])

        # qh: one [33, 376] PSUM tile per head
        for hp in range(NPAIR):
            for hh in range(2):
                qhp = psE.tile([33, 512], F32, tag="qh", name=f"qh{hp}{hh}", bufs=3)
                mm(qhp[:, 0:SQP],
                   wmT[:].rearrange("p (h m) -> p h m", m=33)[:, 2 * hp + hh, :],
                   q1Ta[:])
                evac(qhS[hp][:, SQP * hh : SQP * hh + SQP], qhp[:, 0:SQP], act_ok=True)

        # ---- phase 2 PSUM: v1 + Vh ----
        psum_ctx.close()
        psum_ctx = ExitStack()
        psE2 = psum_ctx.enter_context(tc.tile_pool(name="psE2", bufs=1, space="PSUM"))

        proj_full(psE2, 1, v1Ta)

        # ================= Vh construction =================
        # Vh[jj] layout: [128, 2(j parity), H, 65] fp8; col 64 of each head
        # group is the softmax-denominator ones column (zero on phantom rows).
        VCH = [(0, 7), (462, 7), (924, 6)]  # (col0, heads) 462/462/396 wide
        for j in range(NJ):
            jj, par = (j // 2, j % 2) if ATTNV_DR else (j, 0)
            for (n0, hn) in VCH:
                nw = hn * 66
                vps = psE2.tile([128, 512], F32, tag="vh", name=f"vh{j}_{n0}", bufs=4)
                mm(vps[:, :nw], v1Ta[:, 128 * j : 128 * j + 128], Wv2Ta[:, n0 : n0 + nw])
                dst = Vh[jj][:].rearrange("p (t x) -> p t x", t=2)[
                    :, par, n0 : n0 + nw].rearrange("p (h d) -> p h d", d=66)
                evac(dst, vps[:, :nw].rearrange("p (h d) -> p h d", d=66))

        # ---- switch PSUM pools: attention layout ----
        psum_ctx.close()
        psum_ctx = ExitStack()
        ps_sc = psum_ctx.enter_context(tc.tile_pool(name="ps_sc", bufs=3, space="PSUM"))
        ps_acc = psum_ctx.enter_context(tc.tile_pool(name="ps_acc", bufs=1, space="PSUM"))

        # ================= attention core =================
        # Flat software-pipelined loop over (pair, j): scores two steps
        # ahead so ACT (exp) never starves, including across pair bounds.
        seq = [(hp, j) for hp in range(NPAIR) for j in range(NJ)]
        scs = {}

        def do_scores(idx):
            hp, j = seq[idx]
            sc = ps_sc.tile([128, 1024], F32, tag="sc", name=f"sc{idx}")
            for hh in range(2):
                mm(sc[:, 512 * hh : 512 * hh + SQP],
                   k1aug[:, 128 * j : 128 * j + 128],
                   qhS[hp][:, SQP * hh : SQP * hh + SQP])
            scs[idx] = sc

        do_scores(0)
        do_scores(1)
        accs = {}
        at2s = {}
        for idx, (hp, j) in enumerate(seq):
            jj, par = j // 2, j % 2
            if j == 0:
                accs[hp] = ps_acc.tile([66, 1024], F32, tag="acc", name=f"acc{hp}")
            if par == 0:
                at2s[(hp, jj)] = at2_p.tile(
                    [128, 2 * 2 * SQP], AT2DT, tag="at2", name=f"at2_{hp}_{jj}"
                )
            sc = scs.pop(idx)
            at2 = at2s[(hp, jj)]
            at2v = at2[:].rearrange("p (t h i) -> p t h i", t=2, h=2)
            nc.scalar.activation(
                at2v[:, par, :, :],
                sc[:].rearrange("p (h i) -> p h i", h=2)[:, :, 0:SQP],
                AF.Exp,
                scale=SCALE,
            )
            if idx + 2 < len(seq):
                do_scores(idx + 2)
            if par == 1:
                at2done = at2s.pop((hp, jj))
                at2dv = at2done[:].rearrange("p (t h i) -> p t h i", t=2, h=2)
                Vhv = Vh[jj][:].rearrange("p (t x) -> p t x", t=2)
                for hh in range(2):
                    h0 = (2 * hp + hh) * 66
                    if ATTNV_DR:
                        mm(accs[hp][:, 512 * hh : 512 * hh + SQP],
                           Vhv[:, :, h0 : h0 + 66],
                           at2dv[:, :, hh, :],
                           start=(jj == 0), stop=(jj == NJJ - 1),
                           perf_mode=DR)
                    else:
                        for p2 in range(2):
                            mm(accs[hp][:, 512 * hh : 512 * hh + SQP],
                               Vhv[:, p2, h0 : h0 + 66],
                               at2dv[:, p2, hh, :],
                               start=(jj == 0 and p2 == 0),
                               stop=(jj == NJJ - 1 and p2 == 1))
            if j == NJ - 1:
                # normalize: wvT rows = acc[0:64] * (1/acc[64]) per head
                acc = accs.pop(hp)
                accv = acc[:].rearrange("p (h i) -> p h i", h=2)
                rrs = small_p.tile([1, 2 * SQP], F32R, tag="rrs")
                with nc.allow_low_precision(reason="f32r is bit-identical to f32"):
                    nc.vector.reciprocal(
                        rrs[:].rearrange("p (h i) -> p h i", h=2),
                        accv[64:65, :, 0:SQP],
                    )
                bc_sb = small_p.tile([64, 2 * SQP], F32R, tag="bc_sb", name=f"bcs{hp}")
                nc.gpsimd.partition_broadcast(bc_sb[:], rrs[:])
                for hh in range(2):
                    nc.vector.tensor_mul(
                        wvT[hp][64 * hh : 64 * hh + 64, :],
                        accv[0:64, hh, 0:SQP],
                        bc_sb[:, SQP * hh : SQP * hh + SQP],
                    )

        # ================= output projection =================
        psum_ctx.close()
        psum_ctx = ExitStack()
        psF = psum_ctx.enter_context(tc.tile_pool(name="psF", bufs=3, space="PSUM"))

        o1ps = psF.tile([32, SQP], F32, tag="o1")
        for c in range(10):
            mm(o1ps[:], Wcv[:, 3, c, :], wvT[c][:], start=(c == 0), stop=(c == 9))
        nc.vector.tensor_copy(o1a[0:32, :], o1ps[:])

        ICH = [(i, min(128, SQ - i)) for i in range(0, SQ, 128)]
        OSUB = [(n, min(512, N - n)) for n in range(0, N, 512)]
        for k, (i0, iw) in enumerate(ICH):
            osb = outp.tile([128, N], F32, tag="osb")
            for m, (n0, nw) in enumerate(OSUB):
                fps = psF.tile([128, 512], F32, tag="fps")
                mm(fps[:iw, :nw], o1a[:, i0 : i0 + iw], Wo2Ta[:, n0 : n0 + nw])
                # ACT is idle by now; use it for half the final evacuations
                if (k + m) % 2 == 0:
                    nc.scalar.copy(osb[:iw, n0 : n0 + nw], fps[:iw, :nw])
                else:
                    nc.vector.tensor_copy(osb[:iw, n0 : n0 + nw], fps[:iw, :nw])
            nc.sync.dma_start(out_d[i0 : i0 + iw, :], osb[:iw, :])
        psum_ctx.close()

    nc.compile()
    return nc


_NC_CACHE = {}


def _get_nc(S, SQ=None):
    if S not in _NC_CACHE:
        _NC_CACHE[S] = build_nc(S)
    return _NC_CACHE[S]


def _host_prep(inputs, S):
    """Precompute all weight-derived device tensors in numpy."""
    import ml_dtypes

    f = lambda k: np.asarray(inputs[k], dtype=np.float32)
    Wq1, Wq2, bq = f("Wq1"), f("Wq2"), f("bq")
    Wk1, Wk2, bk = f("Wk1"), f("Wk2"), f("bk")
    Wv1, Wv2, bv = f("Wv1"), f("Wv2"), f("bv")
    Wo1, Wo2, bo = f("Wo1"), f("Wo2"), f("bo")

    Wq2h = Wq2.reshape(H, D, R).transpose(0, 2, 1)  # (H,R,D)
    Wk2h = Wk2.reshape(H, D, R)                     # (H,D,R)
    Wm = Wq2h @ Wk2h                                # (H,R,R)
    bqh = bq.reshape(H, 1, D)
    bkh = bk.reshape(H, D, 1)
    b1 = (Wq2h @ bkh)[:, :, 0]                      # (H,R)
    b2 = (bqh @ Wk2h)[:, 0, :]                      # (H,R)
    b3 = (bqh @ bkh)[:, 0, 0]                       # (H,)

    wmT = np.zeros((33, H, 33), np.float32)
    wmT[0:32, :, 0:32] = Wm.transpose(1, 0, 2)
    wmT[32, :, 0:32] = b2
    wmT[0:32, :, 32] = b1.T
    wmT[32, :, 32] = b3

    Wv2h = Wv2.reshape(H, D, R).transpose(0, 2, 1)  # (H,R,D)
    Wv2Ta = np.zeros((33, H, 66), np.float32)
    Wv2Ta[0:32, :, 0:64] = Wv2h.transpose(1, 0, 2)
    Wv2Ta[32, :, 64] = 1.0

    Wcat = np.zeros((128, 4, 10, 32), np.float32)
    for idx, W in enumerate([Wk1, Wv1, Wq1, Wo1]):
        Wcat[:, idx, :, :] = W.T.reshape(10, 128, 32).transpose(1, 0, 2)

    bo_eff = Wo2 @ (Wo1 @ bv) + bo
    Wo2Ta = np.concatenate([Wo2.T, bo_eff[None, :]], axis=0)  # (33,N)

    NJ = -(-S // 128)
    SP = NJ * 128
    SQ = S // QP
    SQP = SQ + (SQ % 2)
    cbf = np.zeros((1, SP), ml_dtypes.bfloat16)
    cbf[0, 0:S] = 1.0
    cf32 = np.ones((1, SQP), np.float32)

    return {
        "Wcat": np.ascontiguousarray(Wcat.reshape(128, -1).astype(ml_dtypes.bfloat16)),
        "Wv2Ta": np.ascontiguousarray(Wv2Ta.reshape(33, -1).astype(ml_dtypes.bfloat16)),
        "wmT": np.ascontiguousarray(wmT.reshape(33, -1)),
        "Wo2Ta": np.ascontiguousarray(Wo2Ta),
        "cbf": cbf,
        "cf32": cf32,
    }


def kernel(**inputs):
    import ml_dtypes
    from concourse.bass_utils import run_bass_kernel_spmd

    x = np.asarray(inputs["x"], dtype=np.float32)
    B, S, n = x.shape
    assert n == N and B * QP == NCORES
    SQ = S // QP
    NJ = -(-S // 128)
    SP = NJ * 128
    nc = _get_nc(S)

    weights = _host_prep(inputs, S)

    in_maps = []
    for core in range(NCORES):
        b, qi = divmod(core, QP)
        xT = x[b].T  # (N, S)
        xT = np.roll(xT, -SQ * qi, axis=1)
        xTp = np.zeros((N, SP), ml_dtypes.bfloat16)
        xTp[:, 0:S] = xT.astype(ml_dtypes.bfloat16)
        m = {"xT": xTp}
        m.update(weights)
        in_maps.append(m)

    res = run_bass_kernel_spmd(nc, in_maps, core_ids=list(range(NCORES)))
    outs = res.results if hasattr(res, "results") else res

    out = np.zeros((B, S, N), dtype=np.float32)
    for core in range(NCORES):
        b, qi = divmod(core, QP)
        out[b, SQ * qi : SQ * (qi + 1), :] = outs[core]["out"]
    return out


# revision 21
# speedup vs baseline: 1.3570x; 1.0425x over previous
"""Low-rank multi-head attention Bass kernel for Trainium2 (8 NeuronCores).

Sharding: (batch, query-block) data parallel. 8 cores = 2 batches x 4 query
blocks. Each core receives the full (column-rolled) feature-major xT of its
batch and computes full-sequence k1/v1 locally (no collective), plus q1 for
its own 375-query block.

Host-side prep (numpy, inside kernel()):
  - xT = x[b].T rolled so the core's query block is at cols 0:375, padded
    with 36 zero columns to 1536, cast bf16.
  - All weight-derived constants: per-head Wm/b1/b2/b3 folded into wmT,
    Wv2 head layout + ones column (softmax denominator), Wo2.T with
    bo_eff = Wo2@Wo1@bv + bo row (folds both bv and bo), W*1 transposed
    into PE-stationary chunk layout.

Device math (per core, head pair hp = heads 2hp, 2hp+1):
  k1aug = [Wk1 @ xT ; ones]          [33, 1536] bf16 (zero phantom tail)
  v1Ta  = [Wv1 @ xT ; ones]          [33, 1536] bf16
  q1Ta  = [Wq1 @ xT[:, 0:376]; ones] [33, 376]  f32r
  qh_h  = wm_h.T @ q1Ta              [33, 376]  bf16 (all 4 bias terms in)
  scT   = k1aug[:, jblk].T @ qh_h    [128, 376] per j-chunk (bf16 matmul)
  at2   = exp(0.125 * scT)           fp8e4, laid out [128, 2(j), 2(h), 376]
  acc_h = sum_jj DoubleRow(Vh[jj], at2)   [65, 376] f32 (row 64 = denom)
  wvT   = acc[0:64] * (1/acc[64])    bf16 (PE broadcast of reciprocal row)
  o1a   = [Wo1 @ wvT ; ones]         [33, 376] f32r
  out   = o1a.T @ Wo2Ta              (bv/bo folded in row 32)

exp runs exclusively on ACT (the critical path ~97us); evacuations are
spread over DVE and Pool (gpsimd); attnV uses fp8 DoubleRow matmuls
(0.5 cycles/row, K=256 over two j-chunks at once).
"""

import sys

sys.path.insert(0, "/opt/trn_rl_repo")

from contextlib import ExitStack

import numpy as np

import concourse.bass as bass
import concourse.tile as tile
from concourse import bacc
from concourse import mybir

F32 = mybir.dt.float32
F32R = mybir.dt.float32r
BF16 = mybir.dt.bfloat16
FP8 = mybir.dt.float8e4
AF = mybir.ActivationFunctionType
DR = mybir.MatmulPerfMode.DoubleRow

H, D, R, N = 20, 64, 32, 1280
NCORES = 8
QP = 4  # query blocks per batch
SCALE = float(D) ** -0.5  # 0.125

# fp8 DoubleRow for the attn @ V matmuls (0.5 cycles/row, K=256)
ATTNV_DR = True


def build_nc(S):
    SQ = S // QP          # 375
    SQP = SQ + (SQ % 2)   # 376
    NJ = -(-S // 128)     # 12 j-chunks
    assert NJ % 2 == 0
    NJJ = NJ // 2
    SP = NJ * 128         # 1536 padded sequence
    NPAIR = H // 2

    nc = bacc.Bacc("TRN2", target_bir_lowering=False, debug=False, num_devices=NCORES)

    xT_d = nc.dram_tensor("xT", [5, 128, 2 * SP], FP8, kind="ExternalInput")
    Wcat_d = nc.dram_tensor("Wcat", [128, 4 * 5 * 2 * 32], FP8, kind="ExternalInput")
    WcatO_d = nc.dram_tensor("WcatO", [128, 10 * 32], BF16, kind="ExternalInput")
    Wv2Ta_d = nc.dram_tensor("Wv2Ta", [33, H * 66], BF16, kind="ExternalInput")
    cbf_d = nc.dram_tensor("cbf", [1, SP], BF16, kind="ExternalInput")
    cf32_d = nc.dram_tensor("cf32", [1, SQP], F32R, kind="ExternalInput")
    wmT_d = nc.dram_tensor("wmT", [33, H * 33], F32R, kind="ExternalInput")
    Wo2Ta_d = nc.dram_tensor("Wo2Ta", [33, N], F32R, kind="ExternalInput")
    out_d = nc.dram_tensor("out", [SQ, N], F32, kind="ExternalOutput")

    AT2DT = FP8 if ATTNV_DR else BF16

    def mm(out_, lhsT, rhs, **kw):
        nc.tensor.matmul(out_, lhsT, rhs, **kw)

    ev = [0]

    def evac(dst, src, act_ok=False):
        # PSUM evacuations: DVE, alternating with ACT while ACT is idle
        # (GPSIMD cannot access PSUM on TRN2)
        ev[0] += 1
        if act_ok and ev[0] % 2 == 0:
            nc.scalar.copy(dst, src)
        else:
            nc.vector.tensor_copy(dst, src)

    with tile.TileContext(nc) as tc, ExitStack() as ctx:
        wp = ctx.enter_context(tc.tile_pool(name="wp", bufs=1))
        small_p = ctx.enter_context(tc.tile_pool(name="small_p", bufs=4))
        at2_p = ctx.enter_context(tc.tile_pool(name="at2_p", bufs=3))
        outp = ctx.enter_context(tc.tile_pool(name="outp", bufs=2))
        psum_ctx = ExitStack()
        psE = psum_ctx.enter_context(tc.tile_pool(name="psE", bufs=1, space="PSUM"))

        # ---- persistent SBUF tensors ----
        xTs = [wp.tile([128, 2 * SP], FP8, name=f"xT{c}", tag=f"xT{c}") for c in range(5)]
        Wc = wp.tile([128, 4 * 5 * 2 * 32], FP8)
        Wcv = Wc[:].rearrange("p (w c t r) -> p w c t r", w=4, c=5, t=2)
        WcO = wp.tile([128, 10 * 32], BF16)
        WcOv = WcO[:].rearrange("p (c r) -> p c r", c=10)
        Wv2Ta = wp.tile([33, H * 66], BF16)
        wmT = wp.tile([33, H * 33], F32R)
        Wo2Ta = wp.tile([33, N], F32R)
        k1aug = wp.tile([33, SP], BF16)
        v1Ta = wp.tile([33, SP], BF16)
        q1Ta = wp.tile([33, SQP], F32R)
        qhS = [wp.tile([33, 2 * SQP], BF16, name=f"qh{p}", tag=f"qh{p}") for p in range(NPAIR)]
        VHW = ((H * 66 + 15) // 16) * 16  # half-stride must be 16-aligned for dual-fp8 LW
        Vh = [wp.tile([128, 2 * VHW], FP8 if ATTNV_DR else BF16,
                      name=f"Vh{jj}", tag=f"Vh{jj}")
              for jj in range(NJJ if ATTNV_DR else NJ)]
        wvT = [wp.tile([128, SQP], BF16, name=f"wvT{c}", tag=f"wvT{c}") for c in range(10)]
        o1a = wp.tile([33, SQP], F32R)

        # ---- DMA in ----
        nc.sync.dma_start(Wc[:], Wcat_d[:])
        for c in range(5):
            nc.sync.dma_start(xTs[c][:], xT_d[c, :, :])
        nc.sync.dma_start(WcO[:], WcatO_d[:])
        nc.sync.dma_start(Wv2Ta[:], Wv2Ta_d[:])
        nc.sync.dma_start(wmT[:], wmT_d[:])
        nc.sync.dma_start(Wo2Ta[:], Wo2Ta_d[:])

        # ---- ones rows (host-staged constants) ----
        nc.sync.dma_start(k1aug[32:33, :], cbf_d[:])
        nc.sync.dma_start(v1Ta[32:33, :], cbf_d[:])
        nc.sync.dma_start(q1Ta[32:33, :], cf32_d[:])
        nc.sync.dma_start(o1a[32:33, :], cf32_d[:])

        # ================= projections =================
        # k1 full sequence: 3 sub-chunks of 512 cols, accumulated over 10
        # feature chunks; evacuate (cast bf16) as soon as each completes.
        SUBS = [(0, 512), (512, 512), (1024, 512)]

        def proj_full(pool, widx, dstT, act_ok=False):
            tiles = [pool.tile([32, 512], F32, tag=f"pj{s}", name=f"pj{widx}{s}")
                     for s in range(3)]
            for c in range(5):
                xv = xTs[c][:].rearrange("p (t s) -> p t s", t=2)
                for s, (s0, sw) in enumerate(SUBS):
                    mm(tiles[s][:], Wcv[:, widx, c, :, :], xv[:, :, s0 : s0 + sw],
                       start=(c == 0), stop=(c == 4), perf_mode=DR)
            for s, (s0, sw) in enumerate(SUBS):
                evac(dstT[0:32, s0 : s0 + sw], tiles[s][:], act_ok=act_ok)

        proj_full(psE, 0, k1aug, act_ok=True)

        # q1 (own query block only)
        q1ps = psE.tile([32, SQP], F32, tag="q1")
        for c in range(5):
            xv = xTs[c][:].rearrange("p (t s) -> p t s", t=2)
            mm(q1ps[:], Wcv[:, 2, c, :, :], xv[:, :, 0:SQP],
               start=(c == 0), stop=(c == 4), perf_mode=DR)
        nc.vector.tensor_copy(q1Ta[0:32, :], q1ps[:])

        # qh: one [33, 376] PSUM tile per head
        for hp in range(NPAIR):
            for hh in range(2):
                qhp = psE.tile([33, 512], F32, tag="qh", name=f"qh{hp}{hh}", bufs=3)
                mm(qhp[:, 0:SQP],
                   wmT[:].rearrange("p (h m) -> p h m", m=33)[:, 2 * hp + hh, :],
                   q1Ta[:])
                evac(qhS[hp][:, SQP * hh : SQP * hh + SQP], qhp[:, 0:SQP], act_ok=True)

        # ---- phase 2 PSUM: v1 + Vh ----
        psum_ctx.close()
        psum_ctx = ExitStack()
        psE2 = psum_ctx.enter_context(tc.tile_pool(name="psE2", bufs=1, space="PSUM"))

        proj_full(psE2, 1, v1Ta)

        # ================= Vh construction =================
        # Vh[jj] layout: [128, 2(j parity), H, 65] fp8; col 64 of each head
        # group is the softmax-denominator ones column (zero on phantom rows).
        VCH = [(0, 7), (462, 7), (924, 6)]  # (col0, heads) 462/462/396 wide
        for j in range(NJ):
            jj, par = (j // 2, j % 2) if ATTNV_DR else (j, 0)
            for (n0, hn) in VCH:
                nw = hn * 66
                vps = psE2.tile([128, 512], F32, tag="vh", name=f"vh{j}_{n0}", bufs=4)
                mm(vps[:, :nw], v1Ta[:, 128 * j : 128 * j + 128], Wv2Ta[:, n0 : n0 + nw])
                dst = Vh[jj][:].rearrange("p (t x) -> p t x", t=2)[
                    :, par, n0 : n0 + nw].rearrange("p (h d) -> p h d", d=66)
                evac(dst, vps[:, :nw].rearrange("p (h d) -> p h d", d=66))

        # ---- switch PSUM pools: attention layout ----
        psum_ctx.close()
        psum_ctx = ExitStack()
        ps_sc = psum_ctx.enter_context(tc.tile_pool(name="ps_sc", bufs=2, space="PSUM"))
        ps_acc = psum_ctx.enter_context(tc.tile_pool(name="ps_acc", bufs=1, space="PSUM"))
        o1ps = ps_acc.tile([32, SQP], F32, tag="o1ps")

        # ================= attention core =================
        # Flat software-pipelined loop over (pair, j): scores two steps
        # ahead so ACT (exp) never starves, including across pair bounds.
        seq = [(hp, j) for hp in range(NPAIR) for j in range(NJ)]
        scs = {}

        def do_scores(idx):
            hp, j = seq[idx]
            sc = ps_sc.tile([128, 1024], F32, tag="sc", name=f"sc{idx}")
            for hh in range(2):
                mm(sc[:, 512 * hh : 512 * hh + SQP],
                   k1aug[:, 128 * j : 128 * j + 128],
                   qhS[hp][:, SQP * hh : SQP * hh + SQP])
            scs[idx] = sc

        do_scores(0)
        do_scores(1)
        accs = {}
        at2s = {}
        for idx, (hp, j) in enumerate(seq):
            jj, par = j // 2, j % 2
            if j == 0:
                accs[hp] = ps_acc.tile([66, 1024], F32, tag="acc", name=f"acc{hp}")
            if par == 0:
                at2s[(hp, jj)] = at2_p.tile(
                    [128, 2 * 2 * SQP], AT2DT, tag="at2", name=f"at2_{hp}_{jj}"
                )
            sc = scs.pop(idx)
            at2 = at2s[(hp, jj)]
            at2v = at2[:].rearrange("p (t h i) -> p t h i", t=2, h=2)
            nc.scalar.activation(
                at2v[:, par, :, :],
                sc[:].rearrange("p (h i) -> p h i", h=2)[:, :, 0:SQP],
                AF.Exp,
                scale=SCALE,
            )
            if idx + 2 < len(seq):
                do_scores(idx + 2)
            if par == 1:
                at2done = at2s.pop((hp, jj))
                at2dv = at2done[:].rearrange("p (t h i) -> p t h i", t=2, h=2)
                Vhv = Vh[jj][:].rearrange("p (t x) -> p t x", t=2)
                for hh in range(2):
                    h0 = (2 * hp + hh) * 66
                    if ATTNV_DR:
                        mm(accs[hp][:, 512 * hh : 512 * hh + SQP],
                           Vhv[:, :, h0 : h0 + 66],
                           at2dv[:, :, hh, :],
                           start=(jj == 0), stop=(jj == NJJ - 1),
                           perf_mode=DR)
                    else:
                        for p2 in range(2):
                            mm(accs[hp][:, 512 * hh : 512 * hh + SQP],
                               Vhv[:, p2, h0 : h0 + 66],
                               at2dv[:, p2, hh, :],
                               start=(jj == 0 and p2 == 0),
                               stop=(jj == NJJ - 1 and p2 == 1))
            if j == NJ - 1:
                # normalize: wvT rows = acc[0:64] * (1/acc[64]) per head
                acc = accs.pop(hp)
                accv = acc[:].rearrange("p (h i) -> p h i", h=2)
                rrs = small_p.tile([1, 2 * SQP], F32R, tag="rrs")
                with nc.allow_low_precision(reason="f32r is bit-identical to f32"):
                    nc.vector.reciprocal(
                        rrs[:].rearrange("p (h i) -> p h i", h=2),
                        accv[64:65, :, 0:SQP],
                    )
                bc_sb = small_p.tile([64, 2 * SQP], F32R, tag="bc_sb", name=f"bcs{hp}")
                nc.gpsimd.partition_broadcast(bc_sb[:], rrs[:])
                for hh in range(2):
                    nc.vector.tensor_mul(
                        wvT[hp][64 * hh : 64 * hh + 64, :],
                        accv[0:64, hh, 0:SQP],
                        bc_sb[:, SQP * hh : SQP * hh + SQP],
                    )
                # fold this pair into the out-projection rank-32 bottleneck
                mm(o1ps[:], WcOv[:, hp, :], wvT[hp][:],
                   start=(hp == 0), stop=(hp == NPAIR - 1))

        # ================= output projection =================
        nc.vector.tensor_copy(o1a[0:32, :], o1ps[:])
        psum_ctx.close()
        psum_ctx = ExitStack()
        psF = psum_ctx.enter_context(tc.tile_pool(name="psF", bufs=3, space="PSUM"))

        ICH = [(i, min(128, SQ - i)) for i in range(0, SQ, 128)]
        OSUB = [(n, min(512, N - n)) for n in range(0, N, 512)]
        for k, (i0, iw) in enumerate(ICH):
            osb = outp.tile([128, N], F32, tag="osb")
            for m, (n0, nw) in enumerate(OSUB):
                fps = psF.tile([128, 512], F32, tag="fps")
                mm(fps[:iw, :nw], o1a[:, i0 : i0 + iw], Wo2Ta[:, n0 : n0 + nw])
                # ACT is idle by now; use it for half the final evacuations
                if (k + m) % 2 == 0:
                    nc.scalar.copy(osb[:iw, n0 : n0 + nw], fps[:iw, :nw])
                else:
                    nc.vector.tensor_copy(osb[:iw, n0 : n0 + nw], fps[:iw, :nw])
                nc.sync.dma_start(out_d[i0 : i0 + iw, n0 : n0 + nw],
                                  osb[:iw, n0 : n0 + nw])
        psum_ctx.close()

    nc.compile()
    return nc


_NC_CACHE = {}


def _get_nc(S, SQ=None):
    if S not in _NC_CACHE:
        _NC_CACHE[S] = build_nc(S)
    return _NC_CACHE[S]


def _host_prep(inputs, S):
    """Precompute all weight-derived device tensors in numpy."""
    import ml_dtypes

    f = lambda k: np.asarray(inputs[k], dtype=np.float32)
    Wq1, Wq2, bq = f("Wq1"), f("Wq2"), f("bq")
    Wk1, Wk2, bk = f("Wk1"), f("Wk2"), f("bk")
    Wv1, Wv2, bv = f("Wv1"), f("Wv2"), f("bv")
    Wo1, Wo2, bo = f("Wo1"), f("Wo2"), f("bo")

    Wq2h = Wq2.reshape(H, D, R).transpose(0, 2, 1)  # (H,R,D)
    Wk2h = Wk2.reshape(H, D, R)                     # (H,D,R)
    Wm = Wq2h @ Wk2h                                # (H,R,R)
    bqh = bq.reshape(H, 1, D)
    bkh = bk.reshape(H, D, 1)
    b1 = (Wq2h @ bkh)[:, :, 0]                      # (H,R)
    b2 = (bqh @ Wk2h)[:, 0, :]                      # (H,R)
    b3 = (bqh @ bkh)[:, 0, 0]                       # (H,)

    wmT = np.zeros((33, H, 33), np.float32)
    wmT[0:32, :, 0:32] = Wm.transpose(1, 0, 2)
    wmT[32, :, 0:32] = b2
    wmT[0:32, :, 32] = b1.T
    wmT[32, :, 32] = b3

    Wv2h = Wv2.reshape(H, D, R).transpose(0, 2, 1)  # (H,R,D)
    Wv2Ta = np.zeros((33, H, 66), np.float32)
    Wv2Ta[0:32, :, 0:64] = Wv2h.transpose(1, 0, 2)
    Wv2Ta[32, :, 64] = 1.0

    # fp8 stationary weights for the DoubleRow projections:
    # Wcat[p, w, cp, t, r] = W[r, 128*(2cp+t)+p]
    Wcat = np.zeros((128, 4, 5, 2, 32), np.float32)
    for idx, W in enumerate([Wk1, Wv1, Wq1, Wo1]):
        Wcat[:, idx] = W.T.reshape(5, 2, 128, 32).transpose(2, 0, 1, 3)
    # bf16 Wo1 chunks for the o1 projection (reads bf16 wvT)
    WcatO = np.ascontiguousarray(
        Wo1.T.reshape(10, 128, 32).transpose(1, 0, 2).reshape(128, -1)
    ).astype(ml_dtypes.bfloat16)

    bo_eff = Wo2 @ (Wo1 @ bv) + bo
    Wo2Ta = np.concatenate([Wo2.T, bo_eff[None, :]], axis=0)  # (33,N)

    NJ = -(-S // 128)
    SP = NJ * 128
    SQ = S // QP
    SQP = SQ + (SQ % 2)
    cbf = np.zeros((1, SP), ml_dtypes.bfloat16)
    cbf[0, 0:S] = 1.0
    cf32 = np.ones((1, SQP), np.float32)

    return {
        "Wcat": np.ascontiguousarray(Wcat.reshape(128, -1).astype(ml_dtypes.float8_e4m3)),
        "WcatO": WcatO,
        "Wv2Ta": np.ascontiguousarray(Wv2Ta.reshape(33, -1).astype(ml_dtypes.bfloat16)),
        "wmT": np.ascontiguousarray(wmT.reshape(33, -1)),
        "Wo2Ta": np.ascontiguousarray(Wo2Ta),
        "cbf": cbf,
        "cf32": cf32,
    }


def kernel(**inputs):
    import ml_dtypes
    from concourse.bass_utils import run_bass_kernel_spmd

    x = np.asarray(inputs["x"], dtype=np.float32)
    B, S, n = x.shape
    assert n == N and B * QP == NCORES
    SQ = S // QP
    NJ = -(-S // 128)
    SP = NJ * 128
    nc = _get_nc(S)

    weights = _host_prep(inputs, S)

    in_maps = []
    for core in range(NCORES):
        b, qi = divmod(core, QP)
        xT = np.roll(x[b].T, -SQ * qi, axis=1)  # (N, S)
        xTp = np.zeros((N, SP), np.float32)
        xTp[:, 0:S] = xT
        # pair-interleaved fp8 tiles: xT8[cp, p, t, s] = xT[128*(2cp+t)+p, s]
        xT8 = np.ascontiguousarray(
            xTp.reshape(5, 2, 128, SP).transpose(0, 2, 1, 3).reshape(5, 128, 2 * SP)
        ).astype(ml_dtypes.float8_e4m3)
        m = {"xT": xT8}
        m.update(weights)
        in_maps.append(m)

    res = run_bass_kernel_spmd(nc, in_maps, core_ids=list(range(NCORES)))
    outs = res.results if hasattr(res, "results") else res

    out = np.zeros((B, S, N), dtype=np.float32)
    for core in range(NCORES):
        b, qi = divmod(core, QP)
        out[b, SQ * qi : SQ * (qi + 1), :] = outs[core]["out"]
    return out


# revision 24
# speedup vs baseline: 1.6104x; 1.1867x over previous
"""Low-rank multi-head attention Bass kernel for Trainium2 (8 NeuronCores).

Sharding: (batch, query-block) data parallel. 8 cores = 2 batches x 4 query
blocks. Each core receives the full (column-rolled) feature-major xT of its
batch (fp8, pair-interleaved for DoubleRow) and computes full-sequence
k1/v1 locally (no collective), plus q1 for its own 375-query block.

Key algebraic move: attn @ V is reassociated. Instead of materializing
Vh = v1 @ Wv2h (B,H,S,64) we accumulate z_h[r,i] = sum_j v1NT[j,r]*at_h[j,i]
(rank-33 per head, j-contraction on the PE via fp8 DoubleRow), normalize by
the softmax denominator (z row 32, via the ones column of v1NT), and fold
Wv2h INTO the host-precomputed out-projection factor A_h = Wv2h @ Wo1_h.T,
so o1 = sum_h A_h.T @ znorm_h. This removes the whole Vh build (36 matmuls
+ ~22us of PSUM evacuations).

Host-side prep (numpy, inside kernel()):
  - xT rolled/padded/cast fp8e4m3, pair-interleaved [5,128,2,1536].
  - wmT: per-head Wm/b1/b2/b3 folded ([33,H,33]).
  - Wo1v2[r,h,m] = Wv2h @ Wo1_h.T, Wo2Ta with bo_eff row (folds bv+bo).
  - ones rows/cols as tiny constant tensors (k1aug/q1Ta/o1a rows,
    v1NT denominator column with zero phantom rows).

Engine layout: ACT runs exp exclusively (~97us, the critical path); PE does
all matmuls; DVE does PSUM evacuations + reciprocal + normalize muls; Pool
does the partition_broadcast of 1/denom; exp is software-pipelined two
scores ahead across pair boundaries so ACT never starves.
"""

import sys

sys.path.insert(0, "/opt/trn_rl_repo")

from contextlib import ExitStack

import numpy as np

import concourse.bass as bass
import concourse.tile as tile
from concourse import bacc
from concourse import mybir

F32 = mybir.dt.float32
F32R = mybir.dt.float32r
BF16 = mybir.dt.bfloat16
FP8 = mybir.dt.float8e4
AF = mybir.ActivationFunctionType
DR = mybir.MatmulPerfMode.DoubleRow

H, D, R, N = 20, 64, 32, 1280
NCORES = 8
QP = 4  # query blocks per batch
SCALE = float(D) ** -0.5  # 0.125


def build_nc(S):
    SQ = S // QP          # 375
    SQP = SQ + (SQ % 2)   # 376
    NJ = -(-S // 128)     # 12 j-chunks
    assert NJ % 2 == 0
    NJJ = NJ // 2
    SP = NJ * 128         # 1536 padded sequence
    NPAIR = H // 2

    nc = bacc.Bacc("TRN2", target_bir_lowering=False, debug=False, num_devices=NCORES)

    xT_d = nc.dram_tensor("xT", [5, 128, 2 * SP], FP8, kind="ExternalInput")
    Wcat_d = nc.dram_tensor("Wcat", [128, 3 * 5 * 2 * 32], FP8, kind="ExternalInput")
    wmT_d = nc.dram_tensor("wmT", [33, H * 33], F32R, kind="ExternalInput")
    Wo1v2_d = nc.dram_tensor("Wo1v2", [32, H * 32], BF16, kind="ExternalInput")
    Wo2Ta_d = nc.dram_tensor("Wo2Ta", [33, N], F32R, kind="ExternalInput")
    cbf_d = nc.dram_tensor("cbf", [1, SP], BF16, kind="ExternalInput")
    cf32_d = nc.dram_tensor("cf32", [1, SQP], F32R, kind="ExternalInput")
    czc_d = nc.dram_tensor("czc", [NJ, 128], FP8, kind="ExternalInput")
    out_d = nc.dram_tensor("out", [SQ, N], F32, kind="ExternalOutput")

    def mm(out_, lhsT, rhs, **kw):
        nc.tensor.matmul(out_, lhsT, rhs, **kw)

    ev = [0]

    def evac(dst, src, act_ok=False):
        # PSUM evacuations: DVE, alternating with ACT while ACT is idle
        # (GPSIMD cannot access PSUM on TRN2)
        ev[0] += 1
        if act_ok and ev[0] % 2 == 0:
            nc.scalar.copy(dst, src)
        else:
            nc.vector.tensor_copy(dst, src)

    with tile.TileContext(nc) as tc, ExitStack() as ctx:
        wp = ctx.enter_context(tc.tile_pool(name="wp", bufs=1))
        small_p = ctx.enter_context(tc.tile_pool(name="small_p", bufs=4))
        at2_p = ctx.enter_context(tc.tile_pool(name="at2_p", bufs=3))
        outp = ctx.enter_context(tc.tile_pool(name="outp", bufs=2))
        ctxE = ExitStack()
        psE = ctxE.enter_context(tc.tile_pool(name="psE", bufs=1, space="PSUM"))

        # ---- persistent SBUF tensors ----
        xTs = [wp.tile([128, 2 * SP], FP8, name=f"xT{c}", tag=f"xT{c}") for c in range(5)]
        Wc = wp.tile([128, 3 * 5 * 2 * 32], FP8)
        Wcv = Wc[:].rearrange("p (w c t r) -> p w c t r", w=3, c=5, t=2)
        wmT = wp.tile([33, H * 33], F32R)
        Wo1v2 = wp.tile([32, H * 32], BF16)
        Wo2Ta = wp.tile([33, N], F32R)
        k1aug = wp.tile([33, SP], BF16)
        q1Ta = wp.tile([33, SQP], F32R)
        qhS = [wp.tile([33, 2 * SQP], BF16, name=f"qh{p}", tag=f"qh{p}") for p in range(NPAIR)]
        # v1NT[jj]: [128 j, 2 (j-parity), 48] fp8; cols 0:32 = v1 rows, col 32
        # = ones (denominator; zero on phantom rows), 33:48 pad (16-aligned
        # half-stride for dual-fp8 ldweights).
        v1NT = [wp.tile([128, 2 * 48], FP8, name=f"v1NT{jj}", tag=f"v1NT{jj}")
                for jj in range(NJJ)]
        zn = [wp.tile([32, 2 * SQP], BF16, name=f"zn{p}", tag=f"zn{p}") for p in range(NPAIR)]
        o1a = wp.tile([33, SQP], F32R)

        # ---- DMA in ----
        nc.sync.dma_start(Wc[:], Wcat_d[:])
        for c in range(5):
            nc.sync.dma_start(xTs[c][:], xT_d[c, :, :])
        nc.sync.dma_start(wmT[:], wmT_d[:])
        nc.sync.dma_start(Wo1v2[:], Wo1v2_d[:])
        nc.sync.dma_start(Wo2Ta[:], Wo2Ta_d[:])
        nc.sync.dma_start(k1aug[32:33, :], cbf_d[:])
        nc.sync.dma_start(q1Ta[32:33, :], cf32_d[:])
        nc.sync.dma_start(o1a[32:33, :], cf32_d[:])
        for jj in range(NJJ):
            for par in range(2):
                j = 2 * jj + par
                nc.sync.dma_start(
                    v1NT[jj][:].rearrange("p (t r) -> p t r", t=2)[:, par, 32:33],
                    czc_d[j, :].unsqueeze(1),
                )

        # ================= projections =================
        SUBS = [(0, 512), (512, 512), (1024, 512)]

        # k1 full sequence: fp8 DoubleRow over feature pairs
        k1t = [psE.tile([32, 512], F32, tag=f"pj{s}", name=f"pj{s}") for s in range(3)]
        for c in range(5):
            xv = xTs[c][:].rearrange("p (t s) -> p t s", t=2)
            for s, (s0, sw) in enumerate(SUBS):
                mm(k1t[s][:], Wcv[:, 0, c, :, :], xv[:, :, s0 : s0 + sw],
                   start=(c == 0), stop=(c == 4), perf_mode=DR)
        for s, (s0, sw) in enumerate(SUBS):
            evac(k1aug[0:32, s0 : s0 + sw], k1t[s][:], act_ok=True)

        # q1 (own query block only)
        q1ps = psE.tile([32, SQP], F32, tag="q1")
        for c in range(5):
            xv = xTs[c][:].rearrange("p (t s) -> p t s", t=2)
            mm(q1ps[:], Wcv[:, 2, c, :, :], xv[:, :, 0:SQP],
               start=(c == 0), stop=(c == 4), perf_mode=DR)
        nc.vector.tensor_copy(q1Ta[0:32, :], q1ps[:])

        # v1NT: [j, r] layout; j-outer with ping-pong PSUM banks (start=True
        # zeroes a whole 2KB bank, so regions cannot share one bank)
        for j in range(NJ):
            v1ps = psE.tile([128, 512], F32, tag="v1nt", name=f"v1nt{j}", bufs=2)
            for c in range(5):
                xv = xTs[c][:].rearrange("p (t s) -> p t s", t=2)
                mm(v1ps[:, 0:32],
                   xv[:, :, 128 * j : 128 * j + 128],
                   Wcv[:, 1, c, :, :],
                   start=(c == 0), stop=(c == 4), perf_mode=DR)
            evac(v1NT[j // 2][:].rearrange("p (t r) -> p t r", t=2)[:, j % 2, 0:32],
                 v1ps[:, 0:32])

        ctxE.close()

        # ---- attention-phase PSUM: qh 1 + z 2 + sc 4 + o1 1 = 8 banks ----
        ctxO = ExitStack()
        psO = ctxO.enter_context(tc.tile_pool(name="psO", bufs=1, space="PSUM"))
        o1ps = psO.tile([32, SQP], F32, tag="o1ps")
        ctxA = ExitStack()
        qh_pool = ctxA.enter_context(tc.tile_pool(name="qh_pool", bufs=1, space="PSUM"))
        z_pool = ctxA.enter_context(tc.tile_pool(name="z_pool", bufs=1, space="PSUM"))
        ps_sc = ctxA.enter_context(tc.tile_pool(name="ps_sc", bufs=2, space="PSUM"))

        def do_qh(hp, act_ok=False):
            for hh in range(2):
                qhp = qh_pool.tile([33, 512], F32, tag="qh", name=f"qh{hp}{hh}")
                mm(qhp[:, 0:SQP],
                   wmT[:].rearrange("p (h m) -> p h m", m=33)[:, 2 * hp + hh, :],
                   q1Ta[:])
                evac(qhS[hp][:, SQP * hh : SQP * hh + SQP], qhp[:, 0:SQP],
                     act_ok=act_ok)

        do_qh(0, act_ok=True)
        do_qh(1, act_ok=True)

        # ================= attention core =================
        seq = [(hp, j) for hp in range(NPAIR) for j in range(NJ)]
        scs = {}

        def do_scores(idx):
            hp, j = seq[idx]
            sc = ps_sc.tile([128, 1024], F32, tag="sc", name=f"sc{idx}")
            for hh in range(2):
                mm(sc[:, 512 * hh : 512 * hh + SQP],
                   k1aug[:, 128 * j : 128 * j + 128],
                   qhS[hp][:, SQP * hh : SQP * hh + SQP])
            scs[idx] = sc

        do_scores(0)
        do_scores(1)
        zps = {}
        at2s = {}
        for idx, (hp, j) in enumerate(seq):
            jj, par = j // 2, j % 2
            if j == 0:
                zps[hp] = z_pool.tile([33, 1024], F32, tag="zp", name=f"zp{hp}")
            if par == 0:
                at2s[(hp, jj)] = at2_p.tile(
                    [128, 2 * 2 * SQP], FP8, tag="at2", name=f"at2_{hp}_{jj}"
                )
            sc = scs.pop(idx)
            at2 = at2s[(hp, jj)]
            at2v = at2[:].rearrange("p (t h i) -> p t h i", t=2, h=2)
            nc.scalar.activation(
                at2v[:, par, :, :],
                sc[:].rearrange("p (h i) -> p h i", h=2)[:, :, 0:SQP],
                AF.Exp,
                scale=SCALE,
            )
            if idx + 2 < len(seq):
                do_scores(idx + 2)
            # emit next pair's qh while this pair streams (keeps PSUM bounded)
            if j == 5 and hp + 2 < NPAIR:
                do_qh(hp + 2)
            if par == 1:
                at2done = at2s.pop((hp, jj))
                at2dv = at2done[:].rearrange("p (t h i) -> p t h i", t=2, h=2)
                v1v = v1NT[jj][:].rearrange("p (t r) -> p t r", t=2)
                for hh in range(2):
                    mm(zps[hp][:, 512 * hh : 512 * hh + SQP],
                       v1v[:, :, 0:33],
                       at2dv[:, :, hh, :],
                       start=(jj == 0), stop=(jj == NJJ - 1),
                       perf_mode=DR)
            if j == NJ - 1:
                # normalize: znorm_h = z[0:32] * (1/z[32]) and fold into o1
                zp = zps.pop(hp)
                zv = zp[:].rearrange("p (h i) -> p h i", h=2)
                rrs = small_p.tile([1, 2 * SQP], F32R, tag="rrs")
                with nc.allow_low_precision(reason="f32r is bit-identical to f32"):
                    nc.vector.reciprocal(
                        rrs[:].rearrange("p (h i) -> p h i", h=2),
                        zv[32:33, :, 0:SQP],
                    )
                bc_sb = small_p.tile([32, 2 * SQP], F32R, tag="bc_sb", name=f"bcs{hp}")
                nc.gpsimd.partition_broadcast(bc_sb[:], rrs[:])
                for hh in range(2):
                    nc.vector.tensor_mul(
                        zn[hp][:, SQP * hh : SQP * hh + SQP],
                        zv[0:32, hh, 0:SQP],
                        bc_sb[:, SQP * hh : SQP * hh + SQP],
                    )
                for hh in range(2):
                    mm(o1ps[:],
                       Wo1v2[:].rearrange("p (h m) -> p h m", m=32)[:, 2 * hp + hh, :],
                       zn[hp][:, SQP * hh : SQP * hh + SQP],
                       start=(hp == 0 and hh == 0),
                       stop=(hp == NPAIR - 1 and hh == 1))

        # ================= output projection =================
        nc.vector.tensor_copy(o1a[0:32, :], o1ps[:])
        ctxA.close()
        ctxF = ExitStack()
        psF = ctxF.enter_context(tc.tile_pool(name="psF", bufs=2, space="PSUM"))

        ICH = [(i, min(128, SQ - i)) for i in range(0, SQ, 128)]
        OSUB = [(n, min(512, N - n)) for n in range(0, N, 512)]
        for k, (i0, iw) in enumerate(ICH):
            osb = outp.tile([128, N], F32, tag="osb")
            for m, (n0, nw) in enumerate(OSUB):
                fps = psF.tile([128, 512], F32, tag="fps")
                mm(fps[:iw, :nw], o1a[:, i0 : i0 + iw], Wo2Ta[:, n0 : n0 + nw])
                # ACT is idle by now; use it for half the final evacuations
                if (k + m) % 2 == 0:
                    nc.scalar.copy(osb[:iw, n0 : n0 + nw], fps[:iw, :nw])
                else:
                    nc.vector.tensor_copy(osb[:iw, n0 : n0 + nw], fps[:iw, :nw])
                nc.sync.dma_start(out_d[i0 : i0 + iw, n0 : n0 + nw],
                                  osb[:iw, n0 : n0 + nw])
        ctxF.close()
        ctxO.close()

    nc.compile()
    return nc


_NC_CACHE = {}


def _get_nc(S, SQ=None):
    if S not in _NC_CACHE:
        _NC_CACHE[S] = build_nc(S)
    return _NC_CACHE[S]


def _host_prep(inputs, S):
    """Precompute all weight-derived device tensors in numpy."""
    import ml_dtypes

    f = lambda k: np.asarray(inputs[k], dtype=np.float32)
    Wq1, Wq2, bq = f("Wq1"), f("Wq2"), f("bq")
    Wk1, Wk2, bk = f("Wk1"), f("Wk2"), f("bk")
    Wv1, Wv2, bv = f("Wv1"), f("Wv2"), f("bv")
    Wo1, Wo2, bo = f("Wo1"), f("Wo2"), f("bo")

    Wq2h = Wq2.reshape(H, D, R).transpose(0, 2, 1)  # (H,R,D)
    Wk2h = Wk2.reshape(H, D, R)                     # (H,D,R)
    Wm = Wq2h @ Wk2h                                # (H,R,R)
    bqh = bq.reshape(H, 1, D)
    bkh = bk.reshape(H, D, 1)
    b1 = (Wq2h @ bkh)[:, :, 0]                      # (H,R)
    b2 = (bqh @ Wk2h)[:, 0, :]                      # (H,R)
    b3 = (bqh @ bkh)[:, 0, 0]                       # (H,)

    wmT = np.zeros((33, H, 33), np.float32)
    wmT[0:32, :, 0:32] = Wm.transpose(1, 0, 2)
    wmT[32, :, 0:32] = b2
    wmT[0:32, :, 32] = b1.T
    wmT[32, :, 32] = b3

    # A_h = Wv2h @ Wo1_h.T as lhsT[r, (h m)]: o1 += A_h.T @ znorm_h
    Wv2h = Wv2.reshape(H, D, R).transpose(0, 2, 1)  # (H,R,D)
    Wo1h = Wo1.reshape(R, H, D)                     # (m,h,d)
    Wo1v2 = np.einsum("hrd,mhd->rhm", Wv2h, Wo1h)   # (32,H,32)

    # fp8 stationary weights for the DoubleRow projections:
    # Wcat[p, w, cp, t, r] = W[r, 128*(2cp+t)+p] for w in (k1, v1, q1)
    Wcat = np.zeros((128, 3, 5, 2, 32), np.float32)
    for idx, W in enumerate([Wk1, Wv1, Wq1]):
        Wcat[:, idx] = W.T.reshape(5, 2, 128, 32).transpose(2, 0, 1, 3)

    bo_eff = Wo2 @ (Wo1 @ bv) + bo
    Wo2Ta = np.concatenate([Wo2.T, bo_eff[None, :]], axis=0)  # (33,N)

    NJ = -(-S // 128)
    SP = NJ * 128
    SQ = S // QP
    SQP = SQ + (SQ % 2)
    cbf = np.zeros((1, SP), ml_dtypes.bfloat16)
    cbf[0, 0:S] = 1.0
    cf32 = np.ones((1, SQP), np.float32)
    # v1NT denominator column: ones, except zero on phantom rows
    czc = np.zeros((NJ, 128), np.float32)
    for j in range(NJ):
        czc[j, : max(0, min(128, S - 128 * j))] = 1.0

    return {
        "Wcat": np.ascontiguousarray(Wcat.reshape(128, -1).astype(ml_dtypes.float8_e4m3)),
        "wmT": np.ascontiguousarray(wmT.reshape(33, -1)),
        "Wo1v2": np.ascontiguousarray(
            Wo1v2.reshape(32, -1).astype(ml_dtypes.bfloat16)),
        "Wo2Ta": np.ascontiguousarray(Wo2Ta),
        "cbf": cbf,
        "cf32": cf32,
        "czc": czc.astype(ml_dtypes.float8_e4m3),
    }


def kernel(**inputs):
    import ml_dtypes
    from concourse.bass_utils import run_bass_kernel_spmd

    x = np.asarray(inputs["x"], dtype=np.float32)
    B, S, n = x.shape
    assert n == N and B * QP == NCORES
    SQ = S // QP
    NJ = -(-S // 128)
    SP = NJ * 128
    nc = _get_nc(S)

    weights = _host_prep(inputs, S)

    in_maps = []
    for core in range(NCORES):
        b, qi = divmod(core, QP)
        xT = np.roll(x[b].T, -SQ * qi, axis=1)  # (N, S)
        xTp = np.zeros((N, SP), np.float32)
        xTp[:, 0:S] = xT
        # pair-interleaved fp8 tiles: xT8[cp, p, t, s] = xT[128*(2cp+t)+p, s]
        xT8 = np.ascontiguousarray(
            xTp.reshape(5, 2, 128, SP).transpose(0, 2, 1, 3).reshape(5, 128, 2 * SP)
        ).astype(ml_dtypes.float8_e4m3)
        m = {"xT": xT8}
        m.update(weights)
        in_maps.append(m)

    res = run_bass_kernel_spmd(nc, in_maps, core_ids=list(range(NCORES)))
    outs = res.results if hasattr(res, "results") else res

    out = np.zeros((B, S, N), dtype=np.float32)
    for core in range(NCORES):
        b, qi = divmod(core, QP)
        out[b, SQ * qi : SQ * (qi + 1), :] = outs[core]["out"]
    return out
